# revision 50
# baseline (speedup 1.0000x reference)
# Trainium2 Bass kernel for AttentionBlock (conv-qkv + spatial softmax attention
# + 1x1 conv out + residual), data-parallel over batch on 8 NeuronCores.
#
# v2: fp8 everywhere on the PE + exp split ACT/DVE.
#
# Math notes (per image, C=128 channels, N=64*64=4096 pixels):
#   q = conv3x3(x, Wq) + bq; k = conv3x3(x, Wk) + bk; v = conv3x3(x, Wv) + bv
#   A = softmax_j(q_i . k_j / sqrt(C));  out = x + Wo @ (A v) + bo
# Folds:
#   - Wo (1x1) commutes with A: u = conv3x3(x, Wo.Wv), bias (Wo bv + bo) rides
#     the residual since rows of A sum to 1:  xres = x + Wo bv + bo.
#   - bk shifts every logit of a softmax column equally -> dropped.
#   - bq contributes kbq[j] = bq . kconv[:,j], applied as the per-partition
#     bias of the exp (ACT bias AP / DVE tensor_scalar scalar2 AP).
# Engines:
#   - PE (all fp8): S = k^T q (e4m3, 1 col/cy), O += uT-pair @ E-pair and
#     colsum += ones-pair @ E-pair as DoubleRow (K=256 over j-pairs, e5m2 E),
#     convs as DoubleRow over tap-pairs against host-built pair streams.
#   - exp is split: ACT does even j-tiles (spline exp -> e5m2), DVE does odd
#     j-tiles with a one-instruction Schraudolph: i8 = rne(S*A + B[j]) written
#     through an int8 view of the e5m2 tile (bit-pattern = piecewise-linear
#     exp). Means of both halves are matched (c* = -0.22) so even/odd tiles
#     are not systematically rescaled against each other.
#   - colsum -> reciprocal -> DRAM bounce -> partition broadcast; normalize
#     (mul) + residual add run on GpSimd(Pool), final store in fp16.

import numpy as np
import ml_dtypes

try:
    import concourse.bass as bass  # noqa: F401
except ImportError:  # pragma: no cover
    import sys

    sys.path.insert(0, "/opt/trn_rl_repo")

import concourse.bass as bass
import concourse.mybir as mybir
from concourse import bacc
from concourse import tile
from concourse.masks import make_identity

B = 8
C = 128
H = W = 64
N = H * W  # 4096
NTAP = 9
IB = 1024  # attention i-block (columns of the output per PSUM residency)
NIB = N // IB  # 4
NJT = N // 128  # 32 j-tiles
NPAIR = NJT // 2  # 16 j-tile pairs per i-block
SCALE = float(C) ** -0.5
EXP_BIAS = -3.0
# Schraudolph e5m2: i8 = rne(z * 4/ln2 + 60 + CSTAR), z = S*SCALE + EXP_BIAS + kbq*SCALE
ALOG = 4.0 / np.log(2.0)  # 5.770780
CSTAR = -0.22

F32 = mybir.dt.float32
F16 = mybir.dt.float16
F8E4 = mybir.dt.float8e4
F8E5 = mybir.dt.float8e5
I8 = mybir.dt.int8
DR = mybir.MatmulPerfMode.DoubleRow

E4 = ml_dtypes.float8_e4m3
E5 = ml_dtypes.float8_e5m2

# tap pairs for DoubleRow convs: raster taps (dy,dx), pairs (0,1)(2,3)(4,5)(6,7), single 8
TAPS = [(dy, dx) for dy in range(3) for dx in range(3)]

_CACHE = {}


def _build_nc():
    nc = bacc.Bacc(None)

    # pair streams: st_g[c, t, y*64+x] = xpad[c, y+dy_t, x+dx_t] for tap 2g+t
    st_d = [
        nc.dram_tensor(f"st{g}", [C, 2, N], F8E4, kind="ExternalInput")
        for g in range(4)
    ]
    s8_d = nc.dram_tensor("s8", [C, N], F8E4, kind="ExternalInput")
    wq_d = nc.dram_tensor("wq", [C, NTAP, C], F8E4, kind="ExternalInput")
    wk_d = nc.dram_tensor("wk", [C, NTAP, C], F8E4, kind="ExternalInput")
    wu_d = nc.dram_tensor("wu", [C, NTAP, C], F8E4, kind="ExternalInput")
    bq_d = nc.dram_tensor("bq", [C, 1], F8E4, kind="ExternalInput")
    xres_d = nc.dram_tensor("xres", [C, N], F16, kind="ExternalInput")
    out_d = nc.dram_tensor("out", [C, H, W], F16, kind="ExternalOutput")

    with tile.TileContext(nc) as tc:
        with tc.tile_pool(name="persist", bufs=1) as pp:
            st_s = [pp.tile([C, 2, N], F8E4, name=f"st{g}") for g in range(4)]
            s8_s = pp.tile([C, N], F8E4)
            wq_s = pp.tile([C, NTAP, C], F8E4)
            wk_s = pp.tile([C, NTAP, C], F8E4)
            wu_s = pp.tile([C, NTAP, C], F8E4)
            bq_s = pp.tile([C, 1], F8E4)
            xres = pp.tile([C, N], F16)
            kb = pp.tile([C, N], F8E4)
            qb = pp.tile([C, N], F8E4)
            uT = pp.tile([C, NJT, C], F8E4)  # uT[j, jt, c] = u[c, jt*128+j]
            ident = pp.tile([128, 128], F8E4)
            ones2 = pp.tile([128, 2, 16], F8E4)  # [:, :, 0:1] is the DR ones lhsT
            eb_act = pp.tile([C, NJT], F32)  # exp bias per j: kbq*SCALE + EXP_BIAS
            eb_dve = pp.tile([C, NJT], F32)  # schraudolph add const per j
            ones1 = pp.tile([1, 128], F32)  # PE partition-broadcast lhsT

            # --- input DMAs, ordered so phase A can chase the frontier ---
            # weights first (small), then stream row-chunks interleaved.
            # critical path first: wk + first stream chunks gate the first conv.
            # finer chunks early so phase A can chase the DMA frontier.
            nc.scalar.dma_start(wk_s, wk_d[:])
            bounds = [0, 512, 1024, 1536, 2048, 3072, 4096]
            for ch in range(len(bounds) - 1):
                sl = slice(bounds[ch], bounds[ch + 1])
                for g in range(4):
                    q = nc.sync if g % 2 == 0 else nc.scalar
                    q.dma_start(st_s[g][:, :, sl], st_d[g][:, :, sl])
                nc.sync.dma_start(s8_s[:, sl], s8_d[:, sl])
                if ch == 0:
                    nc.scalar.dma_start(wq_s, wq_d[:])
                    nc.scalar.dma_start(wu_s, wu_d[:])
                    nc.scalar.dma_start(bq_s, bq_d[:])
            nc.scalar.dma_start(xres, xres_d[:])

            nc.vector.memset(ones2, 1.0)
            nc.vector.memset(ones1, 1.0)
            make_identity(nc, ident)
            cps = tc.alloc_tile_pool(name="cps", bufs=2, space="PSUM")
            ev = tc.alloc_tile_pool(name="ev", bufs=2)

            # dep-free PE spin while input DMAs land: keeps the HAM activity
            # window busy so the clock gate opens before the first conv.
            for _w in range(28):
                wt = cps.tile([128, 256], F8E4, tag="conv", name="wt")
                nc.tensor.transpose(wt[:, 0:256:2], ident, ident)

            def conv_tile(w_s, dest_ap, t):
                # output cols [t*1024, (t+1)*1024): tap-(pair)-major over two
                # 512-col psum halves so one LDWEIGHTS serves two matmuls
                # (DoubleRow disables FWL; LDW pressure would otherwise stall
                # the PE and bounce the HAM clock gate).
                ps = cps.tile([C, 1024], F32, tag="conv", name="ps")
                # tile 0 runs h-major (all chunk-0 work first) so the PE keeps
                # busy while stream chunk 1 is still in flight; later tiles run
                # g-major so one LDWEIGHTS serves both psum halves.
                hs = [[0], [1]] if t == 0 else [[0, 1]]
                for hg in hs:
                    for g in range(4):
                        for h in hg:
                            sl = slice(t * 1024 + h * 512, t * 1024 + (h + 1) * 512)
                            nc.tensor.matmul(
                                ps[:, h * 512 : (h + 1) * 512],
                                w_s[:, 2 * g : 2 * g + 2, :],
                                st_s[g][:, :, sl],
                                start=(g == 0),
                                stop=False,
                                perf_mode=DR,
                            )
                    for h in hg:
                        sl = slice(t * 1024 + h * 512, t * 1024 + (h + 1) * 512)
                        nc.tensor.matmul(
                            ps[:, h * 512 : (h + 1) * 512],
                            w_s[:, 8, :],
                            s8_s[:, sl],
                            start=False,
                            stop=True,
                        )
                nc.scalar.copy(dest_ap, ps)

            def kbq_tile(t):
                # kbq[j] = bq . kconv[:, j] for the 16 j-tiles of k conv-tile t,
                # then fold into the two per-partition exp-bias tables.
                kq = cps.tile([C, 8], F32, tag="conv", name="kq")
                for jj in range(8):
                    nc.tensor.matmul(
                        kq[:, jj : jj + 1],
                        kb[:, t * 1024 + jj * 128 : t * 1024 + (jj + 1) * 128],
                        bq_s,
                        start=True,
                        stop=True,
                    )
                jt = slice(8 * t, 8 * t + 8)
                nc.vector.tensor_scalar(
                    eb_act[:, jt], kq, SCALE, EXP_BIAS,
                    mybir.AluOpType.mult, mybir.AluOpType.add,
                )
                nc.vector.tensor_scalar(
                    eb_dve[:, jt], kq, SCALE * ALOG,
                    EXP_BIAS * ALOG + 60.0 + CSTAR,
                    mybir.AluOpType.mult, mybir.AluOpType.add,
                )

            def u_tile(t):
                # straight u-conv tile; transposes of the PREVIOUS u tile are
                # emitted first so they never stall the PE on ub's eviction.
                u_transposes(t - 1)
                ub = ev.tile([C, 1024], F8E4, tag="ub", name="ub")
                conv_tile(wu_s, ub[:, :], t)
                _ubs[t] = ub

            _ubs = {}

            def u_transposes(t):
                if t < 0:
                    return
                ub = _ubs.pop(t)
                for s in range(8):
                    # fp8 transpose writes with element step 2; evict on DVE
                    # (ACT carries the conv evictions + attention exp)
                    tp = cps.tile([128, 256], F8E4, tag="conv", name="tp")
                    nc.tensor.transpose(
                        tp[:, 0:256:2], ub[:, s * 128 : (s + 1) * 128], ident
                    )
                    nc.vector.tensor_copy(uT[:, 8 * t + s, :], tp[:, 0:256:2])

            # Phase A: all convs, ordered to chase stream-chunk DMA arrival.
            # kbq/transposes of tile t are emitted after the next big convs so
            # the in-order PE never waits on the ACT evictions they depend on.
            for t in range(4):
                sl = slice(t * 1024, (t + 1) * 1024)
                conv_tile(wk_s, kb[:, sl], t)
                conv_tile(wq_s, qb[:, sl], t)
                u_tile(t)
                kbq_tile(t)
            u_transposes(3)
            ev.release()
            cps.release()

            with tc.tile_pool(name="sps", bufs=4, space="PSUM") as sps, tc.tile_pool(
                name="ops", bufs=1, space="PSUM"
            ) as ops, tc.tile_pool(name="csp", bufs=2, space="PSUM") as csp, tc.tile_pool(
                name="ep", bufs=4
            ) as ep, tc.tile_pool(name="fin", bufs=2) as fin, tc.tile_pool(
                name="dstage", bufs=1, space="DRAM"
            ) as dsp:
                rstage = dsp.tile([N], F32)  # reciprocal row bounced via DRAM

                def s_mm(jt, ib):
                    # two independent [128,512] half-tiles: the exp engines
                    # release each half as soon as its chunk is consumed, so
                    # the next pair's S never waits on a whole-tile release
                    halves = []
                    for h in range(2):
                        sp = sps.tile([C, 512], F32, tag="sp", name="sp")
                        nc.tensor.matmul(
                            sp,
                            kb[:, jt * 128 : (jt + 1) * 128],
                            qb[:, ib * IB + h * 512 : ib * IB + (h + 1) * 512],
                            start=True,
                            stop=True,
                        )
                        halves.append(sp)
                    return halves

                def o_cs_mm(ob, cst, e2, p):
                    # O and colsum accumulation for pair p (emitted late so the
                    # in-order PE never waits on exp); colsum first: its 1-row
                    # output drains instantly, smoothing the issue pipeline
                    for h in range(2):
                        nc.tensor.matmul(
                            cst[h],
                            ones2[:, :, 0:1],
                            e2[:, :, h * 512 : (h + 1) * 512],
                            start=(p == 0),
                            stop=(p == NPAIR - 1),
                            perf_mode=DR,
                        )
                    for h in range(2):
                        nc.tensor.matmul(
                            ob[:, h * 512 : (h + 1) * 512],
                            uT[:, 2 * p : 2 * p + 2, :],
                            e2[:, :, h * 512 : (h + 1) * 512],
                            start=(p == 0),
                            stop=(p == NPAIR - 1),
                            perf_mode=DR,
                        )

                for ib in range(NIB):
                    ob = ops.tile([C, IB], F32, tag="ob", name="ob")
                    cst = [
                        csp.tile([1, 512], F32, tag="cst", name=f"cst{h}")
                        for h in range(2)
                    ]
                    pend = []
                    for p in range(NPAIR):
                        jt0, jt1 = 2 * p, 2 * p + 1
                        sa = s_mm(jt0, ib)
                        sb = s_mm(jt1, ib)
                        e2 = ep.tile([C, 2, IB], F8E5, tag="e2", name="e2")
                        for h in range(2):
                            nc.scalar.activation(
                                e2[:, 0, h * 512 : (h + 1) * 512],
                                sa[h],
                                mybir.ActivationFunctionType.Exp,
                                bias=eb_act[:, jt0 : jt0 + 1],
                                scale=SCALE,
                            )
                        for h in range(2):
                            nc.vector.tensor_scalar(
                                e2[:, 1, h * 512 : (h + 1) * 512].bitcast(I8),
                                sb[h],
                                SCALE * ALOG,
                                eb_dve[:, jt1 : jt1 + 1],
                                mybir.AluOpType.mult,
                                mybir.AluOpType.add,
                            )
                        pend.append((e2, p))
                        depth = 1 if (ib == NIB - 1 and p >= NPAIR - 2) else 3
                        if len(pend) > depth:
                            o_cs_mm(ob, cst, *pend.pop(0))
                    for pe_ in pend:
                        o_cs_mm(ob, cst, *pe_)

                    # epilogue: reciprocal of colsum, bounce to a DRAM row,
                    # partition-broadcast back, normalize + residual on GpSimd.
                    if ib < NIB - 1:
                        # hidden epilogue: DRAM-bounce broadcast + GpSimd norm
                        for h in range(2):
                            rcpt = fin.tile([1, 512], F32, tag="rcp", name="rcp")
                            nc.vector.reciprocal_approx_fast(rcpt, cst[h])
                            nc.sync.dma_start(
                                rstage[ib * IB + h * 512 : ib * IB + (h + 1) * 512],
                                rcpt,
                            )
                        obe = fin.tile([C, IB], F32, tag="obe", name="obe")
                        nc.scalar.copy(obe, ob)
                        csl = slice(ib * IB, (ib + 1) * IB)
                        rb = fin.tile([C, IB], F32, tag="rb", name="rb")
                        # scalar queue: keep the broadcast off the store queue
                        nc.scalar.dma_start(rb, rstage[csl].partition_broadcast(C))
                        nt = fin.tile([C, IB], F32, tag="nt", name="nt")
                        nc.gpsimd.tensor_mul(nt, obe, rb)
                        ot = fin.tile([C, IB], F16, tag="ot", name="ot")
                        nc.gpsimd.tensor_add(ot, nt, xres[:, csl])
                        nc.sync.dma_start(out_d[:, ib * 16 : (ib + 1) * 16, :], ot)
                    else:
                        # exposed epilogue: PE partition-broadcast of 1/colsum
                        # (no DRAM round trip), normalize reads O psum directly
                        for h in range(2):
                            rcpt = fin.tile([1, 512], F32, tag="rcp", name="rcp")
                            nc.vector.reciprocal_approx_fast(rcpt, cst[h])
                            rbp = sps.tile([C, 512], F32, tag="sp", name="rbp")
                            nc.tensor.matmul(rbp, ones1, rcpt, start=True, stop=True)
                            rbs = fin.tile([C, 512], F32, tag="rb2", name="rb2")
                            nc.scalar.copy(rbs, rbp)
                            for qk in range(2):
                                hsl = slice(h * 512 + qk * 256, h * 512 + (qk + 1) * 256)
                                csl = slice(ib * IB + h * 512 + qk * 256,
                                            ib * IB + h * 512 + (qk + 1) * 256)
                                nt = fin.tile([C, 256], F32, tag="nt", name="nt")
                                nc.vector.tensor_mul(
                                    nt, ob[:, hsl], rbs[:, qk * 256 : (qk + 1) * 256]
                                )
                                ot = fin.tile([C, 256], F16, tag="ot", name="ot")
                                nc.gpsimd.tensor_add(ot, nt, xres[:, csl])
                                nc.sync.dma_start(
                                    out_d[
                                        :,
                                        ib * 16 + (h * 8 + qk * 4) : ib * 16
                                        + (h * 8 + (qk + 1) * 4),
                                        :,
                                    ],
                                    ot,
                                )

    nc.finalize()
    return nc


def get_nc():
    if "nc" not in _CACHE:
        _CACHE["nc"] = _build_nc()
    return _CACHE["nc"]


def _prep_host_inputs(x, Wq, bq, Wk, bk, Wv, bv, Wo, bo):
    x = np.ascontiguousarray(np.asarray(x, dtype=np.float32))
    Wq = np.asarray(Wq, dtype=np.float32)
    Wk = np.asarray(Wk, dtype=np.float32)
    Wv = np.asarray(Wv, dtype=np.float64)
    Wo2 = np.asarray(Wo, dtype=np.float64).reshape(C, C)
    bq = np.asarray(bq, dtype=np.float32)
    bv = np.asarray(bv, dtype=np.float64)
    bo = np.asarray(bo, dtype=np.float64)
    # bk dropped: a per-i additive cancels in softmax over j.

    # lhsT layouts: w[c, tap, o] = W[o, c, dy, dx]
    wq = np.ascontiguousarray(Wq.transpose(1, 2, 3, 0).reshape(C, NTAP, C)).astype(E4)
    wk = np.ascontiguousarray(Wk.transpose(1, 2, 3, 0).reshape(C, NTAP, C)).astype(E4)
    Wu = np.einsum("om,mckl->ockl", Wo2, Wv)
    wu = np.ascontiguousarray(
        Wu.transpose(1, 2, 3, 0).reshape(C, NTAP, C).astype(np.float32)
    ).astype(E4)
    bu = (Wo2 @ bv + bo).astype(np.float32)
    bqe = np.ascontiguousarray(bq.reshape(C, 1)).astype(E4)

    # e4m3 padded image -> tap-pair streams (built from the quantized pad so
    # device sees one consistent quantization)
    xpad = np.pad(x, ((0, 0), (0, 0), (1, 1), (1, 1))).astype(E4)

    def win(img, tap):
        dy, dx = TAPS[tap]
        return img[:, dy : dy + H, dx : dx + W].reshape(C, N)

    shared = {"wq": wq, "wk": wk, "wu": wu, "bq": bqe}
    in_maps = []
    for i in range(B):
        m = dict(shared)
        for g in range(4):
            m[f"st{g}"] = np.ascontiguousarray(
                np.stack([win(xpad[i], 2 * g), win(xpad[i], 2 * g + 1)], axis=1)
            )
        m["s8"] = np.ascontiguousarray(win(xpad[i], 8))
        m["xres"] = np.ascontiguousarray(
            (x[i].reshape(C, N) + bu[:, None]).astype(np.float16)
        )
        in_maps.append(m)
    return in_maps


def _run(inputs, trace=False):
    from concourse.bass_utils import run_bass_kernel_spmd

    in_maps = _prep_host_inputs(**inputs)
    nc = get_nc()
    res = run_bass_kernel_spmd(nc, in_maps, core_ids=list(range(B)), trace=trace)
    out = np.stack([np.asarray(res.results[i]["out"]) for i in range(B)])
    return out.reshape(B, C, H, W).astype(np.float32), res


def kernel(**inputs) -> np.ndarray:
    out, _ = _run(inputs, trace=False)
    return out


# revision 52
# speedup vs baseline: 1.0023x; 1.0023x over previous
# Trainium2 Bass kernel for AttentionBlock (conv-qkv + spatial softmax attention
# + 1x1 conv out + residual), data-parallel over batch on 8 NeuronCores.
#
# v2: fp8 everywhere on the PE + exp split ACT/DVE.
#
# Math notes (per image, C=128 channels, N=64*64=4096 pixels):
#   q = conv3x3(x, Wq) + bq; k = conv3x3(x, Wk) + bk; v = conv3x3(x, Wv) + bv
#   A = softmax_j(q_i . k_j / sqrt(C));  out = x + Wo @ (A v) + bo
# Folds:
#   - Wo (1x1) commutes with A: u = conv3x3(x, Wo.Wv), bias (Wo bv + bo) rides
#     the residual since rows of A sum to 1:  xres = x + Wo bv + bo.
#   - bk shifts every logit of a softmax column equally -> dropped.
#   - bq contributes kbq[j] = bq . kconv[:,j], applied as the per-partition
#     bias of the exp (ACT bias AP / DVE tensor_scalar scalar2 AP).
# Engines:
#   - PE (all fp8): S = k^T q (e4m3, 1 col/cy), O += uT-pair @ E-pair and
#     colsum += ones-pair @ E-pair as DoubleRow (K=256 over j-pairs, e5m2 E),
#     convs as DoubleRow over tap-pairs against host-built pair streams.
#   - exp is split: ACT does even j-tiles (spline exp -> e5m2), DVE does odd
#     j-tiles with a one-instruction Schraudolph: i8 = rne(S*A + B[j]) written
#     through an int8 view of the e5m2 tile (bit-pattern = piecewise-linear
#     exp). Means of both halves are matched (c* = -0.22) so even/odd tiles
#     are not systematically rescaled against each other.
#   - colsum -> reciprocal -> DRAM bounce -> partition broadcast; normalize
#     (mul) + residual add run on GpSimd(Pool), final store in fp16.

import numpy as np
import ml_dtypes

try:
    import concourse.bass as bass  # noqa: F401
except ImportError:  # pragma: no cover
    import sys

    sys.path.insert(0, "/opt/trn_rl_repo")

import concourse.bass as bass
import concourse.mybir as mybir
from concourse import bacc
from concourse import tile
from concourse.masks import make_identity

B = 8
C = 128
H = W = 64
N = H * W  # 4096
NTAP = 9
IB = 1024  # attention i-block (columns of the output per PSUM residency)
NIB = N // IB  # 4
NJT = N // 128  # 32 j-tiles
NPAIR = NJT // 2  # 16 j-tile pairs per i-block
SCALE = float(C) ** -0.5
EXP_BIAS = -3.0
# Schraudolph e5m2: i8 = rne(z * 4/ln2 + 60 + CSTAR), z = S*SCALE + EXP_BIAS + kbq*SCALE
ALOG = 4.0 / np.log(2.0)  # 5.770780
CSTAR = -0.22

F32 = mybir.dt.float32
F16 = mybir.dt.float16
F8E4 = mybir.dt.float8e4
F8E5 = mybir.dt.float8e5
I8 = mybir.dt.int8
DR = mybir.MatmulPerfMode.DoubleRow

E4 = ml_dtypes.float8_e4m3
E5 = ml_dtypes.float8_e5m2

# tap pairs for DoubleRow convs: raster taps (dy,dx), pairs (0,1)(2,3)(4,5)(6,7), single 8
TAPS = [(dy, dx) for dy in range(3) for dx in range(3)]

_CACHE = {}


def _build_nc():
    nc = bacc.Bacc(None)

    # pair streams: st_g[c, t, y*64+x] = xpad[c, y+dy_t, x+dx_t] for tap 2g+t
    st_d = [
        nc.dram_tensor(f"st{g}", [C, 2, N], F8E4, kind="ExternalInput")
        for g in range(4)
    ]
    s8_d = nc.dram_tensor("s8", [C, N], F8E4, kind="ExternalInput")
    wq_d = nc.dram_tensor("wq", [C, NTAP, C], F8E4, kind="ExternalInput")
    wk_d = nc.dram_tensor("wk", [C, NTAP, C], F8E4, kind="ExternalInput")
    wu_d = nc.dram_tensor("wu", [C, NTAP, C], F8E4, kind="ExternalInput")
    bq_d = nc.dram_tensor("bq", [C, 1], F8E4, kind="ExternalInput")
    xres_d = nc.dram_tensor("xres", [C, N], F16, kind="ExternalInput")
    out_d = nc.dram_tensor("out", [C, H, W], F16, kind="ExternalOutput")

    with tile.TileContext(nc) as tc:
        with tc.tile_pool(name="persist", bufs=1) as pp:
            st_s = [pp.tile([C, 2, N], F8E4, name=f"st{g}") for g in range(4)]
            s8_s = pp.tile([C, N], F8E4)
            wq_s = pp.tile([C, NTAP, C], F8E4)
            wk_s = pp.tile([C, NTAP, C], F8E4)
            wu_s = pp.tile([C, NTAP, C], F8E4)
            bq_s = pp.tile([C, 1], F8E4)
            xres = pp.tile([C, N], F16)
            kb = pp.tile([C, N], F8E4)
            qb = pp.tile([C, N], F8E4)
            uT = pp.tile([C, NJT, C], F8E4)  # uT[j, jt, c] = u[c, jt*128+j]
            ident = pp.tile([128, 128], F8E4)
            ones2 = pp.tile([128, 2, 16], F8E4)  # [:, :, 0:1] is the DR ones lhsT
            eb_act = pp.tile([C, NJT], F32)  # exp bias per j: kbq*SCALE + EXP_BIAS
            eb_dve = pp.tile([C, NJT], F32)  # schraudolph add const per j
            ones1 = pp.tile([1, 128], F32)  # PE partition-broadcast lhsT

            # --- input DMAs, ordered so phase A can chase the frontier ---
            # weights first (small), then stream row-chunks interleaved.
            # critical path first: wk + first stream chunks gate the first conv.
            # finer chunks early so phase A can chase the DMA frontier.
            nc.scalar.dma_start(wk_s, wk_d[:])
            bounds = [0, 512, 1024, 1536, 2048, 3072, 4096]
            for ch in range(len(bounds) - 1):
                sl = slice(bounds[ch], bounds[ch + 1])
                for g in range(4):
                    q = nc.sync if g % 2 == 0 else nc.scalar
                    q.dma_start(st_s[g][:, :, sl], st_d[g][:, :, sl])
                nc.sync.dma_start(s8_s[:, sl], s8_d[:, sl])
                if ch == 0:
                    nc.scalar.dma_start(wq_s, wq_d[:])
                    nc.scalar.dma_start(wu_s, wu_d[:])
                    nc.scalar.dma_start(bq_s, bq_d[:])
            nc.scalar.dma_start(xres, xres_d[:])

            nc.vector.memset(ones2, 1.0)
            nc.vector.memset(ones1, 1.0)
            make_identity(nc, ident)
            cps = tc.alloc_tile_pool(name="cps", bufs=2, space="PSUM")
            ev = tc.alloc_tile_pool(name="ev", bufs=2)

            # dep-free PE spin while input DMAs land: keeps the HAM activity
            # window busy so the clock gate opens before the first conv.
            for _w in range(48):
                wt = cps.tile([128, 256], F8E4, tag="conv", name="wt")
                nc.tensor.transpose(wt[:, 0:256:2], ident, ident)

            def conv_tile(w_s, dest_ap, t):
                # output cols [t*1024, (t+1)*1024): tap-(pair)-major over two
                # 512-col psum halves so one LDWEIGHTS serves two matmuls
                # (DoubleRow disables FWL; LDW pressure would otherwise stall
                # the PE and bounce the HAM clock gate).
                ps = cps.tile([C, 1024], F32, tag="conv", name="ps")
                # tile 0 runs h-major (all chunk-0 work first) so the PE keeps
                # busy while stream chunk 1 is still in flight; later tiles run
                # g-major so one LDWEIGHTS serves both psum halves.
                hs = [[0], [1]] if t == 0 else [[0, 1]]
                for hg in hs:
                    for g in range(4):
                        for h in hg:
                            sl = slice(t * 1024 + h * 512, t * 1024 + (h + 1) * 512)
                            nc.tensor.matmul(
                                ps[:, h * 512 : (h + 1) * 512],
                                w_s[:, 2 * g : 2 * g + 2, :],
                                st_s[g][:, :, sl],
                                start=(g == 0),
                                stop=False,
                                perf_mode=DR,
                            )
                    for h in hg:
                        sl = slice(t * 1024 + h * 512, t * 1024 + (h + 1) * 512)
                        nc.tensor.matmul(
                            ps[:, h * 512 : (h + 1) * 512],
                            w_s[:, 8, :],
                            s8_s[:, sl],
                            start=False,
                            stop=True,
                        )
                nc.scalar.copy(dest_ap, ps)

            def kbq_tile(t):
                # kbq[j] = bq . kconv[:, j] for the 16 j-tiles of k conv-tile t,
                # then fold into the two per-partition exp-bias tables.
                kq = cps.tile([C, 8], F32, tag="conv", name="kq")
                for jj in range(8):
                    nc.tensor.matmul(
                        kq[:, jj : jj + 1],
                        kb[:, t * 1024 + jj * 128 : t * 1024 + (jj + 1) * 128],
                        bq_s,
                        start=True,
                        stop=True,
                    )
                jt = slice(8 * t, 8 * t + 8)
                nc.vector.tensor_scalar(
                    eb_act[:, jt], kq, SCALE, EXP_BIAS,
                    mybir.AluOpType.mult, mybir.AluOpType.add,
                )
                nc.vector.tensor_scalar(
                    eb_dve[:, jt], kq, SCALE * ALOG,
                    EXP_BIAS * ALOG + 60.0 + CSTAR,
                    mybir.AluOpType.mult, mybir.AluOpType.add,
                )

            def u_tile(t):
                # straight u-conv tile; transposes of the PREVIOUS u tile are
                # emitted first so they never stall the PE on ub's eviction.
                u_transposes(t - 1)
                ub = ev.tile([C, 1024], F8E4, tag="ub", name="ub")
                conv_tile(wu_s, ub[:, :], t)
                _ubs[t] = ub

            _ubs = {}

            def u_transposes(t):
                if t < 0:
                    return
                ub = _ubs.pop(t)
                for s in range(8):
                    # fp8 transpose writes with element step 2; evict on DVE
                    # (ACT carries the conv evictions + attention exp)
                    tp = cps.tile([128, 256], F8E4, tag="conv", name="tp")
                    nc.tensor.transpose(
                        tp[:, 0:256:2], ub[:, s * 128 : (s + 1) * 128], ident
                    )
                    nc.vector.tensor_copy(uT[:, 8 * t + s, :], tp[:, 0:256:2])

            # Phase A: all convs, ordered to chase stream-chunk DMA arrival.
            # kbq/transposes of tile t are emitted after the next big convs so
            # the in-order PE never waits on the ACT evictions they depend on.
            for t in range(4):
                sl = slice(t * 1024, (t + 1) * 1024)
                conv_tile(wk_s, kb[:, sl], t)
                conv_tile(wq_s, qb[:, sl], t)
                u_tile(t)
                kbq_tile(t)
            u_transposes(3)
            ev.release()
            cps.release()

            with tc.tile_pool(name="sps", bufs=4, space="PSUM") as sps, tc.tile_pool(
                name="ops", bufs=1, space="PSUM"
            ) as ops, tc.tile_pool(name="csp", bufs=2, space="PSUM") as csp, tc.tile_pool(
                name="ep", bufs=4
            ) as ep, tc.tile_pool(name="fin", bufs=2) as fin, tc.tile_pool(
                name="dstage", bufs=1, space="DRAM"
            ) as dsp:
                rstage = dsp.tile([N], F32)  # reciprocal row bounced via DRAM

                def s_mm(jt, ib):
                    # two independent [128,512] half-tiles: the exp engines
                    # release each half as soon as its chunk is consumed, so
                    # the next pair's S never waits on a whole-tile release
                    halves = []
                    for h in range(2):
                        sp = sps.tile([C, 512], F32, tag="sp", name="sp")
                        nc.tensor.matmul(
                            sp,
                            kb[:, jt * 128 : (jt + 1) * 128],
                            qb[:, ib * IB + h * 512 : ib * IB + (h + 1) * 512],
                            start=True,
                            stop=True,
                        )
                        halves.append(sp)
                    return halves

                def o_cs_mm(ob, cst, e2, p):
                    # O and colsum accumulation for pair p (emitted late so the
                    # in-order PE never waits on exp); colsum first: its 1-row
                    # output drains instantly, smoothing the issue pipeline
                    for h in range(2):
                        nc.tensor.matmul(
                            cst[h],
                            ones2[:, :, 0:1],
                            e2[:, :, h * 512 : (h + 1) * 512],
                            start=(p == 0),
                            stop=(p == NPAIR - 1),
                            perf_mode=DR,
                        )
                    for h in range(2):
                        nc.tensor.matmul(
                            ob[:, h * 512 : (h + 1) * 512],
                            uT[:, 2 * p : 2 * p + 2, :],
                            e2[:, :, h * 512 : (h + 1) * 512],
                            start=(p == 0),
                            stop=(p == NPAIR - 1),
                            perf_mode=DR,
                        )

                for ib in range(NIB):
                    ob = ops.tile([C, IB], F32, tag="ob", name="ob")
                    cst = [
                        csp.tile([1, 512], F32, tag="cst", name=f"cst{h}")
                        for h in range(2)
                    ]
                    pend = []
                    for p in range(NPAIR):
                        jt0, jt1 = 2 * p, 2 * p + 1
                        sa = s_mm(jt0, ib)
                        sb = s_mm(jt1, ib)
                        e2 = ep.tile([C, 2, IB], F8E5, tag="e2", name="e2")
                        for h in range(2):
                            nc.scalar.activation(
                                e2[:, 0, h * 512 : (h + 1) * 512],
                                sa[h],
                                mybir.ActivationFunctionType.Exp,
                                bias=eb_act[:, jt0 : jt0 + 1],
                                scale=SCALE,
                            )
                        for h in range(2):
                            nc.vector.tensor_scalar(
                                e2[:, 1, h * 512 : (h + 1) * 512].bitcast(I8),
                                sb[h],
                                SCALE * ALOG,
                                eb_dve[:, jt1 : jt1 + 1],
                                mybir.AluOpType.mult,
                                mybir.AluOpType.add,
                            )
                        pend.append((e2, p))
                        depth = 1 if (ib == NIB - 1 and p >= NPAIR - 2) else 3
                        if len(pend) > depth:
                            o_cs_mm(ob, cst, *pend.pop(0))
                    for pe_ in pend:
                        o_cs_mm(ob, cst, *pe_)

                    # epilogue: reciprocal of colsum, bounce to a DRAM row,
                    # partition-broadcast back, normalize + residual on GpSimd.
                    if ib < NIB - 1:
                        # hidden epilogue: DRAM-bounce broadcast + GpSimd norm
                        for h in range(2):
                            rcpt = fin.tile([1, 512], F32, tag="rcp", name="rcp")
                            nc.vector.reciprocal_approx_fast(rcpt, cst[h])
                            nc.sync.dma_start(
                                rstage[ib * IB + h * 512 : ib * IB + (h + 1) * 512],
                                rcpt,
                            )
                        obe = fin.tile([C, IB], F32, tag="obe", name="obe")
                        nc.scalar.copy(obe, ob)
                        csl = slice(ib * IB, (ib + 1) * IB)
                        rb = fin.tile([C, IB], F32, tag="rb", name="rb")
                        # scalar queue: keep the broadcast off the store queue
                        nc.scalar.dma_start(rb, rstage[csl].partition_broadcast(C))
                        nt = fin.tile([C, IB], F32, tag="nt", name="nt")
                        nc.gpsimd.tensor_mul(nt, obe, rb)
                        ot = fin.tile([C, IB], F16, tag="ot", name="ot")
                        nc.gpsimd.tensor_add(ot, nt, xres[:, csl])
                        nc.sync.dma_start(out_d[:, ib * 16 : (ib + 1) * 16, :], ot)
                    else:
                        # exposed epilogue: PE partition-broadcast of 1/colsum
                        # (no DRAM round trip), normalize reads O psum directly
                        for h in range(2):
                            rcpt = fin.tile([1, 512], F32, tag="rcp", name="rcp")
                            nc.vector.reciprocal_approx_fast(rcpt, cst[h])
                            rbp = sps.tile([C, 512], F32, tag="sp", name="rbp")
                            nc.tensor.matmul(rbp, ones1, rcpt, start=True, stop=True)
                            rbs = fin.tile([C, 512], F32, tag="rb2", name="rb2")
                            nc.scalar.copy(rbs, rbp)
                            for qk in range(2):
                                hsl = slice(h * 512 + qk * 256, h * 512 + (qk + 1) * 256)
                                csl = slice(ib * IB + h * 512 + qk * 256,
                                            ib * IB + h * 512 + (qk + 1) * 256)
                                nt = fin.tile([C, 256], F32, tag="nt", name="nt")
                                nc.vector.tensor_mul(
                                    nt, ob[:, hsl], rbs[:, qk * 256 : (qk + 1) * 256]
                                )
                                ot = fin.tile([C, 256], F16, tag="ot", name="ot")
                                nc.gpsimd.tensor_add(ot, nt, xres[:, csl])
                                nc.sync.dma_start(
                                    out_d[
                                        :,
                                        ib * 16 + (h * 8 + qk * 4) : ib * 16
                                        + (h * 8 + (qk + 1) * 4),
                                        :,
                                    ],
                                    ot,
                                )

    nc.finalize()
    return nc


def get_nc():
    if "nc" not in _CACHE:
        _CACHE["nc"] = _build_nc()
    return _CACHE["nc"]


def _prep_host_inputs(x, Wq, bq, Wk, bk, Wv, bv, Wo, bo):
    x = np.ascontiguousarray(np.asarray(x, dtype=np.float32))
    Wq = np.asarray(Wq, dtype=np.float32)
    Wk = np.asarray(Wk, dtype=np.float32)
    Wv = np.asarray(Wv, dtype=np.float64)
    Wo2 = np.asarray(Wo, dtype=np.float64).reshape(C, C)
    bq = np.asarray(bq, dtype=np.float32)
    bv = np.asarray(bv, dtype=np.float64)
    bo = np.asarray(bo, dtype=np.float64)
    # bk dropped: a per-i additive cancels in softmax over j.

    # lhsT layouts: w[c, tap, o] = W[o, c, dy, dx]
    wq = np.ascontiguousarray(Wq.transpose(1, 2, 3, 0).reshape(C, NTAP, C)).astype(E4)
    wk = np.ascontiguousarray(Wk.transpose(1, 2, 3, 0).reshape(C, NTAP, C)).astype(E4)
    Wu = np.einsum("om,mckl->ockl", Wo2, Wv)
    wu = np.ascontiguousarray(
        Wu.transpose(1, 2, 3, 0).reshape(C, NTAP, C).astype(np.float32)
    ).astype(E4)
    bu = (Wo2 @ bv + bo).astype(np.float32)
    bqe = np.ascontiguousarray(bq.reshape(C, 1)).astype(E4)

    # e4m3 padded image -> tap-pair streams (built from the quantized pad so
    # device sees one consistent quantization)
    xpad = np.pad(x, ((0, 0), (0, 0), (1, 1), (1, 1))).astype(E4)

    def win(img, tap):
        dy, dx = TAPS[tap]
        return img[:, dy : dy + H, dx : dx + W].reshape(C, N)

    shared = {"wq": wq, "wk": wk, "wu": wu, "bq": bqe}
    in_maps = []
    for i in range(B):
        m = dict(shared)
        for g in range(4):
            m[f"st{g}"] = np.ascontiguousarray(
                np.stack([win(xpad[i], 2 * g), win(xpad[i], 2 * g + 1)], axis=1)
            )
        m["s8"] = np.ascontiguousarray(win(xpad[i], 8))
        m["xres"] = np.ascontiguousarray(
            (x[i].reshape(C, N) + bu[:, None]).astype(np.float16)
        )
        in_maps.append(m)
    return in_maps


def _run(inputs, trace=False):
    from concourse.bass_utils import run_bass_kernel_spmd

    in_maps = _prep_host_inputs(**inputs)
    nc = get_nc()
    res = run_bass_kernel_spmd(nc, in_maps, core_ids=list(range(B)), trace=trace)
    out = np.stack([np.asarray(res.results[i]["out"]) for i in range(B)])
    return out.reshape(B, C, H, W).astype(np.float32), res


def kernel(**inputs) -> np.ndarray:
    out, _ = _run(inputs, trace=False)
    return out
